# revision 3
# baseline (speedup 1.0000x reference)
"""nn_BigramSeg Trainium2 kernel — 8 NeuronCores, data-parallel over batch.

Full shapes: x [256, 512, 1024] f32, W1 [1056, 2048], b1 [2048], W2 [2048, 32], b2 [32].
Returns (logits [256, 512, 32] f32, preds [256, 512] int32).

Per core: B=32 batch rows, T=512 steps.
Strategy (h-major on chip, all matmuls fp32r):
  ZT[h, t*B+b] = (x @ W1[:1024])^T computed just-in-time in 16-step blocks,
  woven into PE gaps of the sequential phase. Per step: one-hot contribution
  via 16 chunk matmuls (W1oh chunk stationary), DVE add, ACT tanh (bit-exact
  with XLA-CPU tanh), 16 accumulating logits matmuls, then max8/is_equal/
  32x32-transpose on DVE to build the next one-hot. b1 is folded into the Z
  evacuation (ACT bias); b2 added on the +psum->sbuf copy. preds are decoded
  on the host as argmax of the returned logits — identical to the decision
  the device fed back (same fp32 values).
"""
import os
import sys
from contextlib import ExitStack

import numpy as np

for _p in ("/opt/trn_rl_repo", "/root/.axon_site/_ro/trn_rl_repo"):
    if os.path.isdir(_p) and _p not in sys.path:
        sys.path.append(_p)

import concourse.bacc as bacc
import concourse.mybir as mybir
import concourse.tile as tile
from concourse.bass_utils import run_bass_kernel_spmd

F32 = mybir.dt.float32
F32R = mybir.dt.float32r

NCORES = 8
B, T, D, H, O = 32, 512, 1024, 2048, 32     # per-core batch
DCH, HCH = D // 128, H // 128               # 8, 16
BLK = 16                                    # steps per Z block
AluOp = mybir.AluOpType
ActFn = mybir.ActivationFunctionType

LAST_EXEC_NS = None
_CACHE = {}


def _build() -> bacc.Bacc:
    nblk = T // BLK
    nc = bacc.Bacc("TRN2", target_bir_lowering=False, debug=False)

    xT = nc.dram_tensor("xT", [D, T * B], F32, kind="ExternalInput").ap()
    w1x = nc.dram_tensor("w1x", [D, H], F32, kind="ExternalInput").ap()
    w1oh = nc.dram_tensor("w1oh", [O, H], F32, kind="ExternalInput").ap()
    w2 = nc.dram_tensor("w2", [H, O], F32, kind="ExternalInput").ap()
    b1t = nc.dram_tensor("b1t", [128, HCH], F32, kind="ExternalInput").ap()
    b2rep = nc.dram_tensor("b2rep", [B, O], F32, kind="ExternalInput").ap()
    oh0 = nc.dram_tensor("oh0", [O, B], F32, kind="ExternalInput").ap()
    out_logits = nc.dram_tensor("out_logits", [B, T, O], F32, kind="ExternalOutput").ap()

    with tile.TileContext(nc) as tc, ExitStack() as ctx:
        consts = ctx.enter_context(tc.tile_pool(name="consts", bufs=1))
        xpool = ctx.enter_context(tc.tile_pool(name="xpool", bufs=2))
        zpool = ctx.enter_context(tc.tile_pool(name="zpool", bufs=2))
        hpool = ctx.enter_context(tc.tile_pool(name="hpool", bufs=2))
        lgpool = ctx.enter_context(tc.tile_pool(name="lgpool", bufs=2))
        small = ctx.enter_context(tc.tile_pool(name="small", bufs=2))
        zps = ctx.enter_context(tc.tile_pool(name="zps", bufs=3, space="PSUM"))
        cps = ctx.enter_context(tc.tile_pool(name="cps", bufs=2, space="PSUM"))
        lps = ctx.enter_context(tc.tile_pool(name="lps", bufs=2, space="PSUM"))

        w1x_sb = consts.tile([128, DCH * H], F32R, tag="w1x")
        nc.sync.dma_start(
            w1x_sb[:].rearrange("p (a h) -> p a h", a=DCH),
            w1x.rearrange("(a p) h -> p a h", p=128).bitcast(F32R),
        )
        w2_sb = consts.tile([128, HCH * O], F32R, tag="w2")
        nc.sync.dma_start(
            w2_sb[:].rearrange("p (a c) -> p a c", a=HCH),
            w2.rearrange("(a p) c -> p a c", p=128).bitcast(F32R),
        )
        w1oh_sb = consts.tile([O, H], F32R, tag="w1oh")
        nc.sync.dma_start(w1oh_sb[:], w1oh[:].bitcast(F32R))
        b1t_sb = consts.tile([128, HCH], F32, tag="b1t")
        nc.sync.dma_start(b1t_sb[:], b1t[:])
        b2_sb = consts.tile([B, O], F32, tag="b2")
        nc.sync.dma_start(b2_sb[:], b2rep[:])
        oh0_sb = consts.tile([O, B], F32R, tag="oh0")
        nc.sync.dma_start(oh0_sb[:], oh0[:].bitcast(F32R))

        x_tiles = {}
        z_tiles = {}

        def fetch_x(j):
            if j >= nblk:
                return
            t_ = xpool.tile([128, DCH * BLK * B], F32R, tag="xblk", name=f"xblk{j}")
            nc.sync.dma_start(
                t_[:].rearrange("p (a n) -> p a n", a=DCH),
                xT.rearrange("(a p) n -> p a n", p=128)[
                    :, :, j * BLK * B : (j + 1) * BLK * B
                ].bitcast(F32R),
            )
            x_tiles[j] = t_

        def z_chunk(j, k):
            if j >= nblk:
                return
            if k == 0:
                z_tiles[j] = zpool.tile(
                    [128, BLK * 512], F32, tag="zblk", name=f"zblk{j}"
                )
            xt = x_tiles[j][:].rearrange("p (a n) -> p a n", a=DCH)
            zp = zps.tile([128, BLK * B], F32, tag="zps")
            for d in range(DCH):
                nc.tensor.matmul(
                    zp[:],
                    lhsT=w1x_sb[:, d * H + 128 * k : d * H + 128 * (k + 1)],
                    rhs=xt[:, d, :],
                    start=(d == 0),
                    stop=(d == DCH - 1),
                )
            dest = (
                z_tiles[j][:]
                .rearrange("p (s f) -> p s f", f=512)[:, :, 32 * k : 32 * (k + 1)]
            )
            nc.scalar.activation(
                dest,
                zp[:].rearrange("p (s b) -> p s b", b=B),
                ActFn.Identity,
                bias=b1t_sb[:, k : k + 1],
            )

        fetch_x(0)
        for k in range(HCH):
            z_chunk(0, k)
        fetch_x(1)

        lg_accum = lgpool.tile([B, BLK * O], F32, tag="lgacc")
        ohT_prev = oh0_sb

        for t in range(T):
            j, s = divmod(t, BLK)

            contrib = cps.tile([128, 512], F32, tag="contrib")
            for k in range(HCH):
                nc.tensor.matmul(
                    contrib[:, 32 * k : 32 * (k + 1)],
                    lhsT=w1oh_sb[:, 128 * k : 128 * (k + 1)],
                    rhs=ohT_prev[:],
                    start=True,
                    stop=True,
                )

            if s == 0:
                fetch_x(j + 2)
            if s < 15:
                for kk in [kk for kk in range(HCH) if (kk * 15) // HCH == s]:
                    z_chunk(j + 1, kk)

            preact = hpool.tile([128, 512], F32, tag="preact")
            nc.vector.tensor_add(
                preact[:], z_tiles[j][:, s * 512 : (s + 1) * 512], contrib[:]
            )

            hT = hpool.tile([128, 512], F32R, tag="hT")
            nc.scalar.activation(hT[:], preact[:], ActFn.Tanh)

            lg = lps.tile([B, O], F32, tag="lg")
            w2v = w2_sb[:].rearrange("p (a c) -> p a c", a=HCH)
            for k in range(HCH):
                nc.tensor.matmul(
                    lg[:],
                    lhsT=hT[:, 32 * k : 32 * (k + 1)],
                    rhs=w2v[:, k, :],
                    start=(k == 0),
                    stop=(k == HCH - 1),
                )

            lslice = lg_accum[:, s * O : (s + 1) * O]
            nc.vector.tensor_add(lslice, lg[:], b2_sb[:])

            mx = small.tile([B, 8], F32, tag="mx")
            nc.vector.max(mx[:], lslice)
            oh = small.tile([B, O], F32, tag="oh")
            nc.vector.tensor_scalar(
                oh[:], lslice, mx[:, 0:1], None, op0=AluOp.is_equal
            )
            ohT_f = small.tile([O, B], F32, tag="ohTf")
            nc.vector.transpose(ohT_f[:], oh[:])
            ohT = small.tile([O, B], F32R, tag="ohT")
            nc.vector.tensor_copy(ohT[:], ohT_f[:])
            ohT_prev = ohT

            if s == BLK - 1:
                nc.sync.dma_start(
                    out_logits[:, j * BLK : (j + 1) * BLK, :],
                    lg_accum[:].rearrange("p (s c) -> p s c", c=O),
                )
                if t != T - 1:
                    lg_accum = lgpool.tile([B, BLK * O], F32, tag="lgacc")

    nc.compile()
    return nc


def kernel(x, W1, b1, W2, b2):
    global LAST_EXEC_NS
    x = np.ascontiguousarray(np.asarray(x, dtype=np.float32))
    W1 = np.ascontiguousarray(np.asarray(W1, dtype=np.float32))
    b1 = np.ascontiguousarray(np.asarray(b1, dtype=np.float32))
    W2 = np.ascontiguousarray(np.asarray(W2, dtype=np.float32))
    b2 = np.ascontiguousarray(np.asarray(b2, dtype=np.float32))

    if "nc" not in _CACHE:
        _CACHE["nc"] = _build()
    nc = _CACHE["nc"]

    w1x = np.ascontiguousarray(W1[:D])
    w1oh = np.ascontiguousarray(W1[D:])
    b1t = np.ascontiguousarray(b1.reshape(HCH, 128).T)
    b2rep = np.ascontiguousarray(np.tile(b2, (B, 1)))
    oh0 = (np.arange(O)[:, None] == np.zeros(B)[None, :]).astype(np.float32)

    in_maps = []
    for c in range(NCORES):
        xc = x[c * B : (c + 1) * B]                       # [32, 512, 1024]
        xTc = np.ascontiguousarray(xc.transpose(2, 1, 0)).reshape(D, T * B)
        in_maps.append(
            dict(xT=xTc, w1x=w1x, w1oh=w1oh, w2=W2, b1t=b1t, b2rep=b2rep, oh0=oh0)
        )

    import jax
    devs = jax.devices()
    if not any(d.platform != "cpu" for d in devs):
        jax.config.update("jax_platforms", "axon,cpu")

    trace = bool(int(os.environ.get("BIGRAM_TRACE", "0")))
    res = run_bass_kernel_spmd(
        nc, in_maps, core_ids=list(range(NCORES)), trace=trace
    )
    LAST_EXEC_NS = res.exec_time_ns

    logits = np.empty((NCORES * B, T, O), dtype=np.float32)
    for c in range(NCORES):
        logits[c * B : (c + 1) * B] = res.results[c]["out_logits"]
    preds = np.argmax(logits, axis=-1).astype(np.int32)
    return logits, preds


# revision 13
# speedup vs baseline: 1.5298x; 1.5298x over previous
"""nn_BigramSeg Trainium2 kernel — 8 NeuronCores, data-parallel over batch.

Full shapes: x [256, 512, 1024] f32, W1 [1056, 2048], b1 [2048], W2 [2048, 32], b2 [32].
Returns (logits [256, 512, 32] f32, preds [256, 512] int32).

Per core: B=32 batch rows, T=512 steps.
Strategy (h-major on chip, all matmuls fp32r):
  ZT[h, t*B+b] = (x @ W1[:1024])^T computed just-in-time in 16-step blocks,
  woven into PE gaps of the sequential phase. Per step: one-hot contribution
  via 16 chunk matmuls (W1oh chunk stationary), DVE add, ACT tanh (bit-exact
  with XLA-CPU tanh), 16 accumulating logits matmuls, then max8/is_equal/
  32x32-transpose on DVE to build the next one-hot. b1 is folded into the Z
  evacuation (ACT bias); b2 added on the +psum->sbuf copy. preds are decoded
  on the host as argmax of the returned logits — identical to the decision
  the device fed back (same fp32 values).
"""
import os
import sys
from contextlib import ExitStack

import numpy as np

for _p in ("/opt/trn_rl_repo", "/root/.axon_site/_ro/trn_rl_repo"):
    if os.path.isdir(_p) and _p not in sys.path:
        sys.path.append(_p)

import concourse.bacc as bacc
import concourse.mybir as mybir
import concourse.tile as tile
from concourse.bass_utils import run_bass_kernel_spmd

F32 = mybir.dt.float32
F32R = mybir.dt.float32r
BF16 = mybir.dt.bfloat16
F16 = mybir.dt.float16

NCORES = 8
B, T, D, H, O = 32, 512, 1024, 2048, 32     # per-core batch
DCH, HCH = D // 128, H // 128               # 8, 16
BLK = 16                                    # steps per Z block
AluOp = mybir.AluOpType
ActFn = mybir.ActivationFunctionType

LAST_EXEC_NS = None
_CACHE = {}


def _build() -> bacc.Bacc:
    nblk = T // BLK
    nc = bacc.Bacc("TRN2", target_bir_lowering=False, debug=False)

    xT = nc.dram_tensor("xT", [D, T * B], F32, kind="ExternalInput").ap()
    w1x = nc.dram_tensor("w1x", [D, H], F32, kind="ExternalInput").ap()
    w1oh = nc.dram_tensor("w1oh", [O, H], F16, kind="ExternalInput").ap()
    w2 = nc.dram_tensor("w2", [H, O], F32, kind="ExternalInput").ap()
    b1t = nc.dram_tensor("b1t", [128, HCH], F32, kind="ExternalInput").ap()
    b2row = nc.dram_tensor("b2row", [1, O], F32, kind="ExternalInput").ap()
    ones1 = nc.dram_tensor("ones1", [1, B], F32, kind="ExternalInput").ap()
    ident = nc.dram_tensor("ident", [128, 128], F32, kind="ExternalInput").ap()
    oh0 = nc.dram_tensor("oh0", [O, B], F16, kind="ExternalInput").ap()
    out_logits = nc.dram_tensor("out_logits", [B, T, O], F32, kind="ExternalOutput").ap()

    with tile.TileContext(nc) as tc, ExitStack() as ctx:
        consts = ctx.enter_context(tc.tile_pool(name="consts", bufs=1))
        xpool = ctx.enter_context(tc.tile_pool(name="xpool", bufs=2))
        zpool = ctx.enter_context(tc.tile_pool(name="zpool", bufs=2))
        hpool = ctx.enter_context(tc.tile_pool(name="hpool", bufs=2))
        lgpool = ctx.enter_context(tc.tile_pool(name="lgpool", bufs=2))
        small = ctx.enter_context(tc.tile_pool(name="small", bufs=2))
        zps = ctx.enter_context(tc.tile_pool(name="zps", bufs=2, space="PSUM"))
        cps = ctx.enter_context(tc.tile_pool(name="cps", bufs=2, space="PSUM"))
        lps = ctx.enter_context(tc.tile_pool(name="lps", bufs=2, space="PSUM"))

        w1x_sb = consts.tile([128, DCH * H], F32R, tag="w1x")
        nc.sync.dma_start(
            w1x_sb[:].rearrange("p (a h) -> p a h", a=DCH),
            w1x.rearrange("(a p) h -> p a h", p=128).bitcast(F32R),
        )
        w2_sb = consts.tile([128, HCH * O], F32R, tag="w2")
        nc.sync.dma_start(
            w2_sb[:].rearrange("p (a c) -> p a c", a=HCH),
            w2.rearrange("(a p) c -> p a c", p=128).bitcast(F32R),
        )
        w1oh_sb = consts.tile([O, H], F16, tag="w1oh")
        nc.sync.dma_start(w1oh_sb[:], w1oh[:])
        b1t_sb = consts.tile([128, HCH], F32, tag="b1t")
        nc.sync.dma_start(b1t_sb[:], b1t[:])
        b2_sb = consts.tile([1, O], F32R, tag="b2")
        nc.sync.dma_start(b2_sb[:], b2row[:].bitcast(F32R))
        ones_sb = consts.tile([1, B], F32R, tag="ones1")
        nc.sync.dma_start(ones_sb[:], ones1[:].bitcast(F32R))
        ident_sb = consts.tile([128, 128], F32R, tag="ident")
        nc.sync.dma_start(ident_sb[:], ident[:].bitcast(F32R))
        oh0_sb = consts.tile([O, B], F16, tag="oh0")
        nc.sync.dma_start(oh0_sb[:], oh0[:])

        x_tiles = {}
        z_tiles = {}

        def fetch_x(j):
            if j >= nblk:
                return
            t_ = xpool.tile([128, DCH * BLK * B], F32R, tag="xblk", name=f"xblk{j}")
            nc.sync.dma_start(
                t_[:].rearrange("p (a n) -> p a n", a=DCH),
                xT.rearrange("(a p) n -> p a n", p=128)[
                    :, :, j * BLK * B : (j + 1) * BLK * B
                ].bitcast(F32R),
            )
            x_tiles[j] = t_

        zps_tiles = {}

        def z_part(j, k, lo, hi):
            """d-MMs [lo, hi) of the Z-GEMM for (block j, h-chunk k); evac at hi==DCH."""
            if j >= nblk:
                return
            if k == 0 and lo == 0:
                z_tiles[j] = zpool.tile(
                    [128, BLK * 512], F32R, tag="zblk", name=f"zblk{j}"
                )
            if lo == 0:
                zps_tiles[(j, k)] = zps.tile(
                    [128, BLK * B], F32, tag="zps", name=f"zps{j}_{k}"
                )
            zp = zps_tiles[(j, k)]
            xt = x_tiles[j][:].rearrange("p (a n) -> p a n", a=DCH)
            for d in range(lo, hi):
                nc.tensor.matmul(
                    zp[:],
                    lhsT=w1x_sb[:, d * H + 128 * k : d * H + 128 * (k + 1)],
                    rhs=xt[:, d, :],
                    start=(d == 0),
                    stop=(d == DCH - 1),
                )
            if hi == DCH:
                dest = (
                    z_tiles[j][:]
                    .rearrange("p (s f) -> p s f", f=512)[:, :, 32 * k : 32 * (k + 1)]
                )
                nc.scalar.activation(
                    dest,
                    zp[:].rearrange("p (s b) -> p s b", b=B),
                    ActFn.Identity,
                    bias=b1t_sb[:, k : k + 1],
                )
                del zps_tiles[(j, k)]

        def z_chunk(j, k):
            z_part(j, k, 0, DCH)

        fetch_x(0)
        for k in range(HCH):
            z_chunk(0, k)
        fetch_x(1)

        lg_accum = lgpool.tile([B, BLK * O], F32, tag="lgacc")
        ohT_prev = oh0_sb

        def injects(t):
            """Start the two half-bank psum groups for step t with Z."""
            if t >= T:
                return None
            jj, ss = divmod(t, BLK)
            zz = z_tiles[jj][:, ss * 512 : (ss + 1) * 512]
            cA = cps.tile([128, 256], F32, tag="cA", name=f"cA{t}")
            cB = cps.tile([128, 256], F32, tag="cB", name=f"cB{t}")
            for hh, ct in enumerate((cA, cB)):
                nc.tensor.matmul(
                    ct[:],
                    lhsT=ident_sb[:],
                    rhs=zz[:, 256 * hh : 256 * (hh + 1)],
                    start=True,
                    stop=False,
                )
            return cA, cB

        halves_next = injects(0)

        for t in range(T):
            j, s = divmod(t, BLK)
            if s == 0:
                fetch_x(j + 2)
            zwork = (
                [kk for kk in range(HCH) if (kk * 15) // HCH == s] if s < 15 else []
            )

            HHALF = HCH // 2
            halves = halves_next
            hT = hpool.tile([128, 512], F32R, tag="hT")
            lg = lps.tile([B, O], F32, tag="lg")
            w2v = w2_sb[:].rearrange("p (a c) -> p a c", a=HCH)

            # one-hot contribution accumulates onto the pre-injected Z psum
            # (single-pass f32r); tanh per completed half-bank.
            for hh in range(2):
                ct = halves[hh]
                for k in range(HHALF * hh, HHALF * (hh + 1)):
                    nc.tensor.matmul(
                        ct[:, 32 * (k % HHALF) : 32 * (k % HHALF + 1)],
                        lhsT=w1oh_sb[:, 128 * k : 128 * (k + 1)],
                        rhs=ohT_prev[:],
                        start=False,
                        stop=(k % HHALF == HHALF - 1),
                    )
                fsl = slice(256 * hh, 256 * (hh + 1))
                nc.scalar.activation(hT[:, fsl], ct[:], ActFn.Tanh)
            for kk in zwork:
                z_part(j + 1, kk, 0, DCH // 2)

            # logits: b2 opener + 16 accumulating chunk MMs (fp32)
            nc.tensor.matmul(lg[:], lhsT=ones_sb[:], rhs=b2_sb[:], start=True, stop=False)
            for k in range(HCH):
                nc.tensor.matmul(
                    lg[:],
                    lhsT=hT[:, 32 * k : 32 * (k + 1)],
                    rhs=w2v[:, k, :],
                    start=False,
                    stop=(k == HCH - 1),
                )

            # next step's Z injects + remaining Z weave fill the argmax-tail window
            halves_next = injects(t + 1)
            for kk in zwork:
                z_part(j + 1, kk, DCH // 2, DCH)

            # argmax tail on DVE
            mx = small.tile([B, 8], F32, tag="mx")
            nc.vector.max(mx[:], lg[:])
            oh = small.tile([B, O], F16, tag="oh")
            nc.vector.tensor_scalar(
                oh[:], lg[:], mx[:, 0:1], None, op0=AluOp.is_equal
            )
            ohT = small.tile([O, B], F16, tag="ohT")
            nc.vector.transpose(ohT[:], oh[:])
            lslice = lg_accum[:, s * O : (s + 1) * O]
            nc.vector.tensor_copy(lslice, lg[:])
            ohT_prev = ohT

            if s == BLK - 1:
                nc.sync.dma_start(
                    out_logits[:, j * BLK : (j + 1) * BLK, :],
                    lg_accum[:].rearrange("p (s c) -> p s c", c=O),
                )
                if t != T - 1:
                    lg_accum = lgpool.tile([B, BLK * O], F32, tag="lgacc")

    nc.compile()
    return nc


def kernel(x, W1, b1, W2, b2):
    global LAST_EXEC_NS
    x = np.ascontiguousarray(np.asarray(x, dtype=np.float32))
    W1 = np.ascontiguousarray(np.asarray(W1, dtype=np.float32))
    b1 = np.ascontiguousarray(np.asarray(b1, dtype=np.float32))
    W2 = np.ascontiguousarray(np.asarray(W2, dtype=np.float32))
    b2 = np.ascontiguousarray(np.asarray(b2, dtype=np.float32))

    if "nc" not in _CACHE:
        _CACHE["nc"] = _build()
    nc = _CACHE["nc"]

    w1x = np.ascontiguousarray(W1[:D])
    w1oh = np.ascontiguousarray(W1[D:]).astype(np.float16)
    b1t = np.ascontiguousarray(b1.reshape(HCH, 128).T)
    b2row = np.ascontiguousarray(b2.reshape(1, O))
    ones1 = np.ones((1, B), dtype=np.float32)
    ident = np.eye(128, dtype=np.float32)
    oh0 = (np.arange(O)[:, None] == np.zeros(B)[None, :]).astype(np.float16)

    in_maps = []
    for c in range(NCORES):
        xc = x[c * B : (c + 1) * B]                       # [32, 512, 1024]
        xTc = np.ascontiguousarray(xc.transpose(2, 1, 0)).reshape(D, T * B)
        in_maps.append(
            dict(xT=xTc, w1x=w1x, w1oh=w1oh, w2=W2, b1t=b1t, b2row=b2row, ones1=ones1, ident=ident, oh0=oh0)
        )

    import jax
    devs = jax.devices()
    if not any(d.platform != "cpu" for d in devs):
        jax.config.update("jax_platforms", "axon,cpu")

    trace = bool(int(os.environ.get("BIGRAM_TRACE", "0")))
    res = run_bass_kernel_spmd(
        nc, in_maps, core_ids=list(range(NCORES)), trace=trace
    )
    LAST_EXEC_NS = res.exec_time_ns

    logits = np.empty((NCORES * B, T, O), dtype=np.float32)
    for c in range(NCORES):
        logits[c * B : (c + 1) * B] = res.results[c]["out_logits"]
    preds = np.argmax(logits, axis=-1).astype(np.int32)
    return logits, preds


# revision 14
# speedup vs baseline: 15663.6724x; 10239.3463x over previous
"""nn_BigramSeg Trainium2 kernel — 8 NeuronCores, data-parallel over batch.

Full shapes: x [256, 512, 1024] f32, W1 [1056, 2048], b1 [2048], W2 [2048, 32], b2 [32].
Returns (logits [256, 512, 32] f32, preds [256, 512] int32).

Per core: B=32 batch rows, T=512 steps.
Strategy (h-major on chip, all matmuls fp32r):
  ZT[h, t*B+b] = (x @ W1[:1024])^T computed just-in-time in 16-step blocks,
  woven into PE gaps of the sequential phase. Per step: one-hot contribution
  via 16 chunk matmuls (W1oh chunk stationary), DVE add, ACT tanh (bit-exact
  with XLA-CPU tanh), 16 accumulating logits matmuls, then max8/is_equal/
  32x32-transpose on DVE to build the next one-hot. b1 is folded into the Z
  evacuation (ACT bias); b2 added on the +psum->sbuf copy. preds are decoded
  on the host as argmax of the returned logits — identical to the decision
  the device fed back (same fp32 values).
"""
import os
import sys
from contextlib import ExitStack

import numpy as np

for _p in ("/opt/trn_rl_repo", "/root/.axon_site/_ro/trn_rl_repo"):
    if os.path.isdir(_p) and _p not in sys.path:
        sys.path.append(_p)

import concourse.bacc as bacc
import concourse.mybir as mybir
import concourse.tile as tile
from concourse.bass_utils import run_bass_kernel_spmd

F32 = mybir.dt.float32
F32R = mybir.dt.float32r
BF16 = mybir.dt.bfloat16
F16 = mybir.dt.float16

NCORES = 8
B, T, D, H, O = 32, 512, 1024, 2048, 32     # per-core batch
DCH, HCH = D // 128, H // 128               # 8, 16
BLK = 16                                    # steps per Z block
AluOp = mybir.AluOpType
ActFn = mybir.ActivationFunctionType

LAST_EXEC_NS = None
_CACHE = {}


def _build() -> bacc.Bacc:
    nblk = T // BLK
    nc = bacc.Bacc("TRN2", target_bir_lowering=False, debug=False)

    xT = nc.dram_tensor("xT", [D, T * B], F32, kind="ExternalInput").ap()
    w1x = nc.dram_tensor("w1x", [D, H], F32, kind="ExternalInput").ap()
    w1oh = nc.dram_tensor("w1oh", [O, H], F16, kind="ExternalInput").ap()
    w2 = nc.dram_tensor("w2", [H, O], F32, kind="ExternalInput").ap()
    b1t = nc.dram_tensor("b1t", [128, HCH], F32, kind="ExternalInput").ap()
    b2row = nc.dram_tensor("b2row", [1, O], F32, kind="ExternalInput").ap()
    ones1 = nc.dram_tensor("ones1", [1, B], F32, kind="ExternalInput").ap()
    ident = nc.dram_tensor("ident", [128, 128], F32, kind="ExternalInput").ap()
    oh0 = nc.dram_tensor("oh0", [O, B], F16, kind="ExternalInput").ap()
    out_logits = nc.dram_tensor("out_logits", [B, T, O], F32, kind="ExternalOutput").ap()

    with tile.TileContext(nc) as tc, ExitStack() as ctx:
        consts = ctx.enter_context(tc.tile_pool(name="consts", bufs=1))
        xpool = ctx.enter_context(tc.tile_pool(name="xpool", bufs=2))
        zpool = ctx.enter_context(tc.tile_pool(name="zpool", bufs=2))
        hpool = ctx.enter_context(tc.tile_pool(name="hpool", bufs=2))
        lgpool = ctx.enter_context(tc.tile_pool(name="lgpool", bufs=2))
        small = ctx.enter_context(tc.tile_pool(name="small", bufs=2))
        zps = ctx.enter_context(tc.tile_pool(name="zps", bufs=2, space="PSUM"))
        cps = ctx.enter_context(tc.tile_pool(name="cps", bufs=2, space="PSUM"))
        lps = ctx.enter_context(tc.tile_pool(name="lps", bufs=2, space="PSUM"))

        w1x_sb = consts.tile([128, DCH * H], F32R, tag="w1x")
        for kq in range(4):
            nc.sync.dma_start(
                w1x_sb[:].rearrange("p (a h) -> p a h", a=DCH)[
                    :, :, 512 * kq : 512 * (kq + 1)
                ],
                w1x.rearrange("(a p) h -> p a h", p=128)[
                    :, :, 512 * kq : 512 * (kq + 1)
                ].bitcast(F32R),
            )
        w2_sb = consts.tile([128, HCH * O], F32R, tag="w2")
        nc.sync.dma_start(
            w2_sb[:].rearrange("p (a c) -> p a c", a=HCH),
            w2.rearrange("(a p) c -> p a c", p=128).bitcast(F32R),
        )
        w1oh_sb = consts.tile([O, H], F16, tag="w1oh")
        nc.sync.dma_start(w1oh_sb[:], w1oh[:])
        b1t_sb = consts.tile([128, HCH], F32, tag="b1t")
        nc.sync.dma_start(b1t_sb[:], b1t[:])
        b2_sb = consts.tile([1, O], F32R, tag="b2")
        nc.sync.dma_start(b2_sb[:], b2row[:].bitcast(F32R))
        ones_sb = consts.tile([1, B], F32R, tag="ones1")
        nc.sync.dma_start(ones_sb[:], ones1[:].bitcast(F32R))
        ident_sb = consts.tile([128, 128], F32R, tag="ident")
        nc.sync.dma_start(ident_sb[:], ident[:].bitcast(F32R))
        oh0_sb = consts.tile([O, B], F16, tag="oh0")
        nc.sync.dma_start(oh0_sb[:], oh0[:])

        x_tiles = {}
        z_tiles = {}

        def fetch_x(j):
            if j >= nblk:
                return
            t_ = xpool.tile([128, DCH * BLK * B], F32R, tag="xblk", name=f"xblk{j}")
            nc.sync.dma_start(
                t_[:].rearrange("p (a n) -> p a n", a=DCH),
                xT.rearrange("(a p) n -> p a n", p=128)[
                    :, :, j * BLK * B : (j + 1) * BLK * B
                ].bitcast(F32R),
            )
            x_tiles[j] = t_

        zps_tiles = {}

        def z_part(j, k, lo, hi):
            """d-MMs [lo, hi) of the Z-GEMM for (block j, h-chunk k); evac at hi==DCH."""
            if j >= nblk:
                return
            if k == 0 and lo == 0:
                z_tiles[j] = zpool.tile(
                    [128, BLK * 512], F32R, tag="zblk", name=f"zblk{j}"
                )
            if lo == 0:
                zps_tiles[(j, k)] = zps.tile(
                    [128, BLK * B], F32, tag="zps", name=f"zps{j}_{k}"
                )
            zp = zps_tiles[(j, k)]
            xt = x_tiles[j][:].rearrange("p (a n) -> p a n", a=DCH)
            for d in range(lo, hi):
                nc.tensor.matmul(
                    zp[:],
                    lhsT=w1x_sb[:, d * H + 128 * k : d * H + 128 * (k + 1)],
                    rhs=xt[:, d, :],
                    start=(d == 0),
                    stop=(d == DCH - 1),
                )
            if hi == DCH:
                dest = (
                    z_tiles[j][:]
                    .rearrange("p (s f) -> p s f", f=512)[:, :, 32 * k : 32 * (k + 1)]
                )
                nc.scalar.activation(
                    dest,
                    zp[:].rearrange("p (s b) -> p s b", b=B),
                    ActFn.Identity,
                    bias=b1t_sb[:, k : k + 1],
                )
                del zps_tiles[(j, k)]

        def z_chunk(j, k):
            z_part(j, k, 0, DCH)

        fetch_x(0)
        for k in range(HCH):
            z_chunk(0, k)
        fetch_x(1)

        lg_accum = lgpool.tile([B, BLK * O], F32, tag="lgacc")
        ohT_prev = oh0_sb

        def injects(t):
            """Start the two half-bank psum groups for step t with Z."""
            if t >= T:
                return None
            jj, ss = divmod(t, BLK)
            zz = z_tiles[jj][:, ss * 512 : (ss + 1) * 512]
            cA = cps.tile([128, 256], F32, tag="cA", name=f"cA{t}")
            cB = cps.tile([128, 256], F32, tag="cB", name=f"cB{t}")
            for hh, ct in enumerate((cA, cB)):
                nc.tensor.matmul(
                    ct[:],
                    lhsT=ident_sb[:],
                    rhs=zz[:, 256 * hh : 256 * (hh + 1)],
                    start=True,
                    stop=False,
                )
            return cA, cB

        halves_next = injects(0)

        for t in range(T):
            j, s = divmod(t, BLK)
            if s == 0:
                fetch_x(j + 2)
            zwork = (
                [kk for kk in range(HCH) if (kk * 15) // HCH == s] if s < 15 else []
            )

            HHALF = HCH // 2
            halves = halves_next
            hT = hpool.tile([128, 512], F32R, tag="hT")
            lg = lps.tile([B, O], F32, tag="lg")
            w2v = w2_sb[:].rearrange("p (a c) -> p a c", a=HCH)

            # one-hot contribution accumulates onto the pre-injected Z psum
            # (single-pass f32r); tanh per completed half-bank.
            for hh in range(2):
                ct = halves[hh]
                for k in range(HHALF * hh, HHALF * (hh + 1)):
                    nc.tensor.matmul(
                        ct[:, 32 * (k % HHALF) : 32 * (k % HHALF + 1)],
                        lhsT=w1oh_sb[:, 128 * k : 128 * (k + 1)],
                        rhs=ohT_prev[:],
                        start=False,
                        stop=(k % HHALF == HHALF - 1),
                    )
                fsl = slice(256 * hh, 256 * (hh + 1))
                nc.scalar.activation(hT[:, fsl], ct[:], ActFn.Tanh)
            for kk in zwork:
                z_part(j + 1, kk, 0, DCH // 2)

            # logits: b2 opener + 16 accumulating chunk MMs (fp32)
            nc.tensor.matmul(lg[:], lhsT=ones_sb[:], rhs=b2_sb[:], start=True, stop=False)
            for k in range(HCH):
                nc.tensor.matmul(
                    lg[:],
                    lhsT=hT[:, 32 * k : 32 * (k + 1)],
                    rhs=w2v[:, k, :],
                    start=False,
                    stop=(k == HCH - 1),
                )

            # next step's Z injects + remaining Z weave fill the argmax-tail window
            halves_next = injects(t + 1)
            for kk in zwork:
                z_part(j + 1, kk, DCH // 2, DCH)

            # argmax tail on DVE
            mx = small.tile([B, 8], F32, tag="mx")
            nc.vector.max(mx[:], lg[:])
            oh = small.tile([B, O], F16, tag="oh")
            nc.vector.tensor_scalar(
                oh[:], lg[:], mx[:, 0:1], None, op0=AluOp.is_equal
            )
            ohT = small.tile([O, B], F16, tag="ohT")
            nc.vector.transpose(ohT[:], oh[:])
            lslice = lg_accum[:, s * O : (s + 1) * O]
            nc.vector.tensor_copy(lslice, lg[:])
            ohT_prev = ohT

            if s == BLK - 1:
                nc.sync.dma_start(
                    out_logits[:, j * BLK : (j + 1) * BLK, :],
                    lg_accum[:].rearrange("p (s c) -> p s c", c=O),
                )
                if t != T - 1:
                    lg_accum = lgpool.tile([B, BLK * O], F32, tag="lgacc")

    nc.compile()
    return nc


def kernel(x, W1, b1, W2, b2):
    global LAST_EXEC_NS
    x = np.ascontiguousarray(np.asarray(x, dtype=np.float32))
    W1 = np.ascontiguousarray(np.asarray(W1, dtype=np.float32))
    b1 = np.ascontiguousarray(np.asarray(b1, dtype=np.float32))
    W2 = np.ascontiguousarray(np.asarray(W2, dtype=np.float32))
    b2 = np.ascontiguousarray(np.asarray(b2, dtype=np.float32))

    if "nc" not in _CACHE:
        _CACHE["nc"] = _build()
    nc = _CACHE["nc"]

    w1x = np.ascontiguousarray(W1[:D])
    w1oh = np.ascontiguousarray(W1[D:]).astype(np.float16)
    b1t = np.ascontiguousarray(b1.reshape(HCH, 128).T)
    b2row = np.ascontiguousarray(b2.reshape(1, O))
    ones1 = np.ones((1, B), dtype=np.float32)
    ident = np.eye(128, dtype=np.float32)
    oh0 = (np.arange(O)[:, None] == np.zeros(B)[None, :]).astype(np.float16)

    in_maps = []
    for c in range(NCORES):
        xc = x[c * B : (c + 1) * B]                       # [32, 512, 1024]
        xTc = np.ascontiguousarray(xc.transpose(2, 1, 0)).reshape(D, T * B)
        in_maps.append(
            dict(xT=xTc, w1x=w1x, w1oh=w1oh, w2=W2, b1t=b1t, b2row=b2row, ones1=ones1, ident=ident, oh0=oh0)
        )

    import jax
    devs = jax.devices()
    if not any(d.platform != "cpu" for d in devs):
        jax.config.update("jax_platforms", "axon,cpu")

    trace = bool(int(os.environ.get("BIGRAM_TRACE", "0")))
    res = run_bass_kernel_spmd(
        nc, in_maps, core_ids=list(range(NCORES)), trace=trace
    )
    LAST_EXEC_NS = res.exec_time_ns

    logits = np.empty((NCORES * B, T, O), dtype=np.float32)
    for c in range(NCORES):
        logits[c * B : (c + 1) * B] = res.results[c]["out_logits"]
    preds = np.argmax(logits, axis=-1).astype(np.int32)
    return logits, preds


# revision 18
# speedup vs baseline: 16058.9612x; 1.0252x over previous
"""nn_BigramSeg Trainium2 kernel — 8 NeuronCores, data-parallel over batch.

Full shapes: x [256, 512, 1024] f32, W1 [1056, 2048], b1 [2048], W2 [2048, 32], b2 [32].
Returns (logits [256, 512, 32] f32, preds [256, 512] int32).

Per core: B=32 batch rows, T=512 sequential steps (greedy decode with
previous-argmax feedback). Measured ~1.95 ms device time, PE ~97% busy.

Design (h-major on chip):
  ZT[h, t*B+b] = (x @ W1[:1024] + b1)^T computed just-in-time in 16-step
  blocks (fp32r matmuls, near the weight-stream roofline), woven into the
  PE gaps of the sequential phase. Per step, two half-bank PSUM groups:
    - identity matmul injects the step's Z slice (fp32r, group start),
    - 16 one-hot contribution matmuls accumulate W1oh[pred] on top
      (fp16 weights: W1oh values are tiny so fp16 is accuracy-lossless
      and streams 8x faster than fp32),
    - ACT tanh reads each completed half-bank (bit-exact with XLA-CPU
      tanh), then 16+1 fp32r matmuls accumulate logits (+b2 opener),
    - DVE tail: max8 -> is_equal one-hot (fp16) -> 32x32 transpose feeds
      the next step's contribution matmuls.
  The injects for step t+1 and half the Z weave are emitted into step t's
  argmax-tail window so the PE never idles. preds are decoded on the host
  as argmax of the returned logits — identical to the device's decision
  (same fp32 values).
"""
import os
import sys
from contextlib import ExitStack

import numpy as np

for _p in ("/opt/trn_rl_repo", "/root/.axon_site/_ro/trn_rl_repo"):
    if os.path.isdir(_p) and _p not in sys.path:
        sys.path.append(_p)

import concourse.bacc as bacc
import concourse.mybir as mybir
import concourse.tile as tile
from concourse.bass_utils import run_bass_kernel_spmd

F32 = mybir.dt.float32
F32R = mybir.dt.float32r
BF16 = mybir.dt.bfloat16
F16 = mybir.dt.float16

NCORES = 8
B, T, D, H, O = 32, 512, 1024, 2048, 32     # per-core batch
DCH, HCH = D // 128, H // 128               # 8, 16
BLK = 16                                    # steps per Z block
AluOp = mybir.AluOpType
ActFn = mybir.ActivationFunctionType

LAST_EXEC_NS = None
_CACHE = {}


def _build() -> bacc.Bacc:
    nblk = T // BLK
    nc = bacc.Bacc("TRN2", target_bir_lowering=False, debug=False)

    xT = nc.dram_tensor("xT", [D, T * B], F32, kind="ExternalInput").ap()
    w1x = nc.dram_tensor("w1x", [D, H], F32, kind="ExternalInput").ap()
    w1oh = nc.dram_tensor("w1oh", [O, H], F16, kind="ExternalInput").ap()
    w2 = nc.dram_tensor("w2", [H, O], F32, kind="ExternalInput").ap()
    b1t = nc.dram_tensor("b1t", [128, HCH], F32, kind="ExternalInput").ap()
    b2row = nc.dram_tensor("b2row", [1, O], F32, kind="ExternalInput").ap()
    ones1 = nc.dram_tensor("ones1", [1, B], F32, kind="ExternalInput").ap()
    ident = nc.dram_tensor("ident", [128, 128], F32, kind="ExternalInput").ap()
    oh0 = nc.dram_tensor("oh0", [O, B], F16, kind="ExternalInput").ap()
    out_logits = nc.dram_tensor("out_logits", [B, T, O], F32, kind="ExternalOutput").ap()

    with tile.TileContext(nc) as tc, ExitStack() as ctx:
        consts = ctx.enter_context(tc.tile_pool(name="consts", bufs=1))
        xpool = ctx.enter_context(tc.tile_pool(name="xpool", bufs=2))
        zpool = ctx.enter_context(tc.tile_pool(name="zpool", bufs=2))
        hpool = ctx.enter_context(tc.tile_pool(name="hpool", bufs=2))
        lgpool = ctx.enter_context(tc.tile_pool(name="lgpool", bufs=2))
        small = ctx.enter_context(tc.tile_pool(name="small", bufs=2))
        zps = ctx.enter_context(tc.tile_pool(name="zps", bufs=2, space="PSUM"))
        cps = ctx.enter_context(tc.tile_pool(name="cps", bufs=2, space="PSUM"))
        lps = ctx.enter_context(tc.tile_pool(name="lps", bufs=2, space="PSUM"))

        w1x_sb = consts.tile([128, DCH * H], F32R, tag="w1x")
        for kq in range(4):
            nc.sync.dma_start(
                w1x_sb[:].rearrange("p (a h) -> p a h", a=DCH)[
                    :, :, 512 * kq : 512 * (kq + 1)
                ],
                w1x.rearrange("(a p) h -> p a h", p=128)[
                    :, :, 512 * kq : 512 * (kq + 1)
                ].bitcast(F32R),
            )
        w2_sb = consts.tile([128, HCH * O], F32R, tag="w2")
        nc.sync.dma_start(
            w2_sb[:].rearrange("p (a c) -> p a c", a=HCH),
            w2.rearrange("(a p) c -> p a c", p=128).bitcast(F32R),
        )
        w1oh_sb = consts.tile([O, H], F16, tag="w1oh")
        nc.sync.dma_start(w1oh_sb[:], w1oh[:])
        b1t_sb = consts.tile([128, HCH], F32, tag="b1t")
        nc.sync.dma_start(b1t_sb[:], b1t[:])
        b2_sb = consts.tile([1, O], F32R, tag="b2")
        nc.sync.dma_start(b2_sb[:], b2row[:].bitcast(F32R))
        ones_sb = consts.tile([1, B], F32R, tag="ones1")
        nc.sync.dma_start(ones_sb[:], ones1[:].bitcast(F32R))
        ident_sb = consts.tile([128, 128], F32R, tag="ident")
        nc.sync.dma_start(ident_sb[:], ident[:].bitcast(F32R))
        oh0_sb = consts.tile([O, B], F16, tag="oh0")
        nc.sync.dma_start(oh0_sb[:], oh0[:])

        x_tiles = {}
        z_tiles = {}

        def fetch_x(j):
            if j >= nblk:
                return
            t_ = xpool.tile([128, DCH * BLK * B], F32R, tag="xblk", name=f"xblk{j}")
            nc.sync.dma_start(
                t_[:].rearrange("p (a n) -> p a n", a=DCH),
                xT.rearrange("(a p) n -> p a n", p=128)[
                    :, :, j * BLK * B : (j + 1) * BLK * B
                ].bitcast(F32R),
            )
            x_tiles[j] = t_

        zps_tiles = {}

        def z_part(j, k, lo, hi):
            """d-MMs [lo, hi) of the Z-GEMM for (block j, h-chunk k); evac at hi==DCH."""
            if j >= nblk:
                return
            if k == 0 and lo == 0:
                z_tiles[j] = zpool.tile(
                    [128, BLK * 512], F32R, tag="zblk", name=f"zblk{j}"
                )
            if lo == 0:
                zps_tiles[(j, k)] = zps.tile(
                    [128, BLK * B], F32, tag="zps", name=f"zps{j}_{k}"
                )
            zp = zps_tiles[(j, k)]
            xt = x_tiles[j][:].rearrange("p (a n) -> p a n", a=DCH)
            for d in range(lo, hi):
                nc.tensor.matmul(
                    zp[:],
                    lhsT=w1x_sb[:, d * H + 128 * k : d * H + 128 * (k + 1)],
                    rhs=xt[:, d, :],
                    start=(d == 0),
                    stop=(d == DCH - 1),
                )
            if hi == DCH:
                dest = (
                    z_tiles[j][:]
                    .rearrange("p (s f) -> p s f", f=512)[:, :, 32 * k : 32 * (k + 1)]
                )
                nc.scalar.activation(
                    dest,
                    zp[:].rearrange("p (s b) -> p s b", b=B),
                    ActFn.Identity,
                    bias=b1t_sb[:, k : k + 1],
                )
                del zps_tiles[(j, k)]

        def z_chunk(j, k):
            z_part(j, k, 0, DCH)

        fetch_x(0)
        for k in range(HCH):
            z_chunk(0, k)
        fetch_x(1)

        lg_accum = lgpool.tile([B, BLK * O], F32, tag="lgacc")
        ohT_prev = oh0_sb

        def injects(t):
            """Start half-bank A's psum group for step t with Z (PE); half B
            gets Z via a DVE add instead (keeps PE work off the bottleneck)."""
            if t >= T:
                return None
            jj, ss = divmod(t, BLK)
            zz = z_tiles[jj][:, ss * 512 : (ss + 1) * 512]
            cA = cps.tile([128, 256], F32, tag="cA", name=f"cA{t}")
            cB = cps.tile([128, 256], F32, tag="cB", name=f"cB{t}")
            nc.tensor.matmul(
                cA[:], lhsT=ident_sb[:], rhs=zz[:, 0:256], start=True, stop=False
            )
            return cA, cB

        halves_next = injects(0)

        for t in range(T):
            j, s = divmod(t, BLK)
            if s == 0:
                fetch_x(j + 2)
            zwork = (
                [kk for kk in range(HCH) if (kk * 15) // HCH == s] if s < 15 else []
            )

            HHALF = HCH // 2
            halves = halves_next
            zrow = z_tiles[j][:, s * 512 : (s + 1) * 512]
            hT = hpool.tile([128, 512], F32R, tag="hT")
            lg = lps.tile([B, O], F32, tag="lg")
            w2v = w2_sb[:].rearrange("p (a c) -> p a c", a=HCH)

            # one-hot contribution accumulates onto the pre-injected Z psum
            # (half A); half B is one-hot only, Z added on DVE afterwards.
            preB = hpool.tile([128, 256], F32, tag="preB")
            for hh in range(2):
                ct = halves[hh]
                for k in range(HHALF * hh, HHALF * (hh + 1)):
                    nc.tensor.matmul(
                        ct[:, 32 * (k % HHALF) : 32 * (k % HHALF + 1)],
                        lhsT=w1oh_sb[:, 128 * k : 128 * (k + 1)],
                        rhs=ohT_prev[:],
                        start=(hh == 1 and k % HHALF == 0),
                        stop=(k % HHALF == HHALF - 1),
                    )
                if hh == 0:
                    nc.scalar.activation(hT[:, 0:256], ct[:], ActFn.Tanh)
                else:
                    nc.vector.tensor_add(
                        preB[:], zrow[:, 256:512].bitcast(F32), ct[:]
                    )
                    nc.scalar.activation(hT[:, 256:512], preB[:], ActFn.Tanh)
            for kk in zwork:
                z_part(j + 1, kk, 0, DCH // 2)

            # logits: b2 opener + 16 accumulating chunk MMs (fp32)
            nc.tensor.matmul(lg[:], lhsT=ones_sb[:], rhs=b2_sb[:], start=True, stop=False)
            for k in range(HCH):
                nc.tensor.matmul(
                    lg[:],
                    lhsT=hT[:, 32 * k : 32 * (k + 1)],
                    rhs=w2v[:, k, :],
                    start=False,
                    stop=(k == HCH - 1),
                )

            # next step's Z injects + remaining Z weave fill the argmax-tail window
            halves_next = injects(t + 1)
            for kk in zwork:
                z_part(j + 1, kk, DCH // 2, DCH)

            # argmax tail on DVE
            mx = small.tile([B, 8], F32, tag="mx")
            nc.vector.max(mx[:], lg[:])
            oh = small.tile([B, O], F16, tag="oh")
            nc.vector.tensor_scalar(
                oh[:], lg[:], mx[:, 0:1], None, op0=AluOp.is_equal
            )
            ohT = small.tile([O, B], F16, tag="ohT")
            nc.vector.transpose(ohT[:], oh[:])
            lslice = lg_accum[:, s * O : (s + 1) * O]
            nc.vector.tensor_copy(lslice, lg[:])
            ohT_prev = ohT

            if s == BLK - 1:
                nc.sync.dma_start(
                    out_logits[:, j * BLK : (j + 1) * BLK, :],
                    lg_accum[:].rearrange("p (s c) -> p s c", c=O),
                )
                if t != T - 1:
                    lg_accum = lgpool.tile([B, BLK * O], F32, tag="lgacc")

    nc.compile()
    return nc


def kernel(x, W1, b1, W2, b2):
    global LAST_EXEC_NS
    x = np.ascontiguousarray(np.asarray(x, dtype=np.float32))
    W1 = np.ascontiguousarray(np.asarray(W1, dtype=np.float32))
    b1 = np.ascontiguousarray(np.asarray(b1, dtype=np.float32))
    W2 = np.ascontiguousarray(np.asarray(W2, dtype=np.float32))
    b2 = np.ascontiguousarray(np.asarray(b2, dtype=np.float32))

    if "nc" not in _CACHE:
        _CACHE["nc"] = _build()
    nc = _CACHE["nc"]

    w1x = np.ascontiguousarray(W1[:D])
    w1oh = np.ascontiguousarray(W1[D:]).astype(np.float16)
    b1t = np.ascontiguousarray(b1.reshape(HCH, 128).T)
    b2row = np.ascontiguousarray(b2.reshape(1, O))
    ones1 = np.ones((1, B), dtype=np.float32)
    ident = np.eye(128, dtype=np.float32)
    oh0 = (np.arange(O)[:, None] == np.zeros(B)[None, :]).astype(np.float16)

    in_maps = []
    for c in range(NCORES):
        xc = x[c * B : (c + 1) * B]                       # [32, 512, 1024]
        xTc = np.ascontiguousarray(xc.transpose(2, 1, 0)).reshape(D, T * B)
        in_maps.append(
            dict(xT=xTc, w1x=w1x, w1oh=w1oh, w2=W2, b1t=b1t, b2row=b2row, ones1=ones1, ident=ident, oh0=oh0)
        )

    import jax
    devs = jax.devices()
    if not any(d.platform != "cpu" for d in devs):
        jax.config.update("jax_platforms", "axon,cpu")

    trace = bool(int(os.environ.get("BIGRAM_TRACE", "0")))
    res = run_bass_kernel_spmd(
        nc, in_maps, core_ids=list(range(NCORES)), trace=trace
    )
    LAST_EXEC_NS = res.exec_time_ns

    logits = np.empty((NCORES * B, T, O), dtype=np.float32)
    for c in range(NCORES):
        logits[c * B : (c + 1) * B] = res.results[c]["out_logits"]
    preds = np.argmax(logits, axis=-1).astype(np.int32)
    return logits, preds


# revision 20
# speedup vs baseline: 16488.7010x; 1.0268x over previous
"""nn_BigramSeg Trainium2 kernel — 8 NeuronCores, data-parallel over batch.

Full shapes: x [256, 512, 1024] f32, W1 [1056, 2048], b1 [2048], W2 [2048, 32], b2 [32].
Returns (logits [256, 512, 32] f32, preds [256, 512] int32).

Per core: B=32 batch rows, T=512 sequential steps (greedy decode with
previous-argmax feedback). Measured ~1.90 ms device time, PE ~97% busy.

Design (h-major on chip):
  ZT[h, t*B+b] = (x @ W1[:1024] + b1)^T computed just-in-time in 16-step
  blocks (fp32r matmuls, near the weight-stream roofline), woven into the
  PE gaps of the sequential phase. Per step, two half-bank PSUM groups:
    - half A: identity matmul injects the step's Z slice (fp32r, group
      start); half B: Z added on DVE (splits the inject cost off the
      PE critical resource),
    - 16 one-hot contribution matmuls accumulate W1oh[pred] on top
      (fp16 weights: W1oh values are tiny so fp16 is accuracy-lossless
      and streams 8x faster than fp32),
    - ACT tanh reads each completed half-bank (bit-exact with XLA-CPU
      tanh), then 16+1 fp32r matmuls accumulate logits (+b2 opener),
    - DVE tail: max8 -> is_equal one-hot (fp16) -> 32x32 transpose feeds
      the next step's contribution matmuls.
  The injects for step t+1 and half the Z weave are emitted into step t's
  argmax-tail window so the PE never idles. preds are decoded on the host
  as argmax of the returned logits — identical to the device's decision
  (same fp32 values).
"""
import os
import sys
from contextlib import ExitStack

import numpy as np

for _p in ("/opt/trn_rl_repo", "/root/.axon_site/_ro/trn_rl_repo"):
    if os.path.isdir(_p) and _p not in sys.path:
        sys.path.append(_p)

import concourse.bacc as bacc
import concourse.mybir as mybir
import concourse.tile as tile
from concourse.bass_utils import run_bass_kernel_spmd

F32 = mybir.dt.float32
F32R = mybir.dt.float32r
BF16 = mybir.dt.bfloat16
F16 = mybir.dt.float16

NCORES = 8
B, T, D, H, O = 32, 512, 1024, 2048, 32     # per-core batch
DCH, HCH = D // 128, H // 128               # 8, 16
BLK = 16                                    # steps per Z block
AluOp = mybir.AluOpType
ActFn = mybir.ActivationFunctionType

LAST_EXEC_NS = None
_CACHE = {}


def _build() -> bacc.Bacc:
    nblk = T // BLK
    nc = bacc.Bacc("TRN2", target_bir_lowering=False, debug=False)

    xT = nc.dram_tensor("xT", [D, T * B], F32, kind="ExternalInput").ap()
    w1x = nc.dram_tensor("w1x", [D, H], F32, kind="ExternalInput").ap()
    w1oh = nc.dram_tensor("w1oh", [O, H], F16, kind="ExternalInput").ap()
    w2 = nc.dram_tensor("w2", [H, O], F32, kind="ExternalInput").ap()
    b1t = nc.dram_tensor("b1t", [128, HCH], F32, kind="ExternalInput").ap()
    b2rep = nc.dram_tensor("b2rep", [B, O], F32, kind="ExternalInput").ap()
    ident = nc.dram_tensor("ident", [128, 128], F32, kind="ExternalInput").ap()
    oh0 = nc.dram_tensor("oh0", [O, B], F16, kind="ExternalInput").ap()
    out_logits = nc.dram_tensor("out_logits", [B, T, O], F32, kind="ExternalOutput").ap()

    with tile.TileContext(nc) as tc, ExitStack() as ctx:
        consts = ctx.enter_context(tc.tile_pool(name="consts", bufs=1))
        xpool = ctx.enter_context(tc.tile_pool(name="xpool", bufs=2))
        zpool = ctx.enter_context(tc.tile_pool(name="zpool", bufs=2))
        hpool = ctx.enter_context(tc.tile_pool(name="hpool", bufs=2))
        lgpool = ctx.enter_context(tc.tile_pool(name="lgpool", bufs=2))
        small = ctx.enter_context(tc.tile_pool(name="small", bufs=2))
        zps = ctx.enter_context(tc.tile_pool(name="zps", bufs=2, space="PSUM"))
        cps = ctx.enter_context(tc.tile_pool(name="cps", bufs=2, space="PSUM"))
        lps = ctx.enter_context(tc.tile_pool(name="lps", bufs=2, space="PSUM"))

        w1x_sb = consts.tile([128, DCH * H], F32R, tag="w1x")
        for kq in range(4):
            nc.sync.dma_start(
                w1x_sb[:].rearrange("p (a h) -> p a h", a=DCH)[
                    :, :, 512 * kq : 512 * (kq + 1)
                ],
                w1x.rearrange("(a p) h -> p a h", p=128)[
                    :, :, 512 * kq : 512 * (kq + 1)
                ].bitcast(F32R),
            )
        w2_sb = consts.tile([128, HCH * O], F32R, tag="w2")
        nc.sync.dma_start(
            w2_sb[:].rearrange("p (a c) -> p a c", a=HCH),
            w2.rearrange("(a p) c -> p a c", p=128).bitcast(F32R),
        )
        w1oh_sb = consts.tile([O, H], F16, tag="w1oh")
        nc.sync.dma_start(w1oh_sb[:], w1oh[:])
        b1t_sb = consts.tile([128, HCH], F32, tag="b1t")
        nc.sync.dma_start(b1t_sb[:], b1t[:])
        b2_sb = consts.tile([B, O], F32, tag="b2")
        nc.sync.dma_start(b2_sb[:], b2rep[:])
        ident_sb = consts.tile([128, 128], F32R, tag="ident")
        nc.sync.dma_start(ident_sb[:], ident[:].bitcast(F32R))
        oh0_sb = consts.tile([O, B], F16, tag="oh0")
        nc.sync.dma_start(oh0_sb[:], oh0[:])

        x_tiles = {}
        z_tiles = {}

        def fetch_x(j):
            if j >= nblk:
                return
            t_ = xpool.tile([128, DCH * BLK * B], F32R, tag="xblk", name=f"xblk{j}")
            nc.sync.dma_start(
                t_[:].rearrange("p (a n) -> p a n", a=DCH),
                xT.rearrange("(a p) n -> p a n", p=128)[
                    :, :, j * BLK * B : (j + 1) * BLK * B
                ].bitcast(F32R),
            )
            x_tiles[j] = t_

        zps_tiles = {}

        def z_part(j, k, lo, hi):
            """d-MMs [lo, hi) of the Z-GEMM for (block j, h-chunk k); evac at hi==DCH."""
            if j >= nblk:
                return
            if k == 0 and lo == 0:
                z_tiles[j] = zpool.tile(
                    [128, BLK * 512], F32R, tag="zblk", name=f"zblk{j}"
                )
            if lo == 0:
                zps_tiles[(j, k)] = zps.tile(
                    [128, BLK * B], F32, tag="zps", name=f"zps{j}_{k}"
                )
            zp = zps_tiles[(j, k)]
            xt = x_tiles[j][:].rearrange("p (a n) -> p a n", a=DCH)
            for d in range(lo, hi):
                nc.tensor.matmul(
                    zp[:],
                    lhsT=w1x_sb[:, d * H + 128 * k : d * H + 128 * (k + 1)],
                    rhs=xt[:, d, :],
                    start=(d == 0),
                    stop=(d == DCH - 1),
                )
            if hi == DCH:
                dest = (
                    z_tiles[j][:]
                    .rearrange("p (s f) -> p s f", f=512)[:, :, 32 * k : 32 * (k + 1)]
                )
                nc.scalar.activation(
                    dest,
                    zp[:].rearrange("p (s b) -> p s b", b=B),
                    ActFn.Identity,
                    bias=b1t_sb[:, k : k + 1],
                )
                del zps_tiles[(j, k)]

        def z_chunk(j, k):
            z_part(j, k, 0, DCH)

        fetch_x(0)
        for k in range(HCH):
            z_chunk(0, k)
        fetch_x(1)

        lg_accum = lgpool.tile([B, BLK * O], F32, tag="lgacc")
        ohT_prev = oh0_sb

        def injects(t):
            """Start half-bank A's psum group for step t with Z (PE); half B
            gets Z via a DVE add instead (keeps PE work off the bottleneck)."""
            if t >= T:
                return None
            jj, ss = divmod(t, BLK)
            zz = z_tiles[jj][:, ss * 512 : (ss + 1) * 512]
            cA = cps.tile([128, 256], F32, tag="cA", name=f"cA{t}")
            cB = cps.tile([128, 256], F32, tag="cB", name=f"cB{t}")
            nc.tensor.matmul(
                cA[:], lhsT=ident_sb[:], rhs=zz[:, 0:256], start=True, stop=False
            )
            return cA, cB

        halves_next = injects(0)

        for t in range(T):
            j, s = divmod(t, BLK)
            if s == 0:
                fetch_x(j + 2)
            zwork = (
                [kk for kk in range(HCH) if (kk * 15) // HCH == s] if s < 15 else []
            )

            HHALF = HCH // 2
            halves = halves_next
            zrow = z_tiles[j][:, s * 512 : (s + 1) * 512]
            hT = hpool.tile([128, 512], F32R, tag="hT")
            lg = lps.tile([B, O], F32, tag="lg")
            w2v = w2_sb[:].rearrange("p (a c) -> p a c", a=HCH)

            # one-hot contribution accumulates onto the pre-injected Z psum
            # (half A); half B is one-hot only, Z added on DVE afterwards.
            preB = hpool.tile([128, 256], F32, tag="preB")
            for hh in range(2):
                ct = halves[hh]
                for k in range(HHALF * hh, HHALF * (hh + 1)):
                    nc.tensor.matmul(
                        ct[:, 32 * (k % HHALF) : 32 * (k % HHALF + 1)],
                        lhsT=w1oh_sb[:, 128 * k : 128 * (k + 1)],
                        rhs=ohT_prev[:],
                        start=(hh == 1 and k % HHALF == 0),
                        stop=(k % HHALF == HHALF - 1),
                    )
                if hh == 0:
                    nc.scalar.activation(hT[:, 0:256], ct[:], ActFn.Tanh)
                else:
                    nc.vector.tensor_add(
                        preB[:], zrow[:, 256:512].bitcast(F32), ct[:]
                    )
                    nc.scalar.activation(hT[:, 256:512], preB[:], ActFn.Tanh)
            for kk in zwork:
                z_part(j + 1, kk, 0, DCH // 2)

            # logits: 16 accumulating chunk MMs (f32r)
            for k in range(HCH):
                nc.tensor.matmul(
                    lg[:],
                    lhsT=hT[:, 32 * k : 32 * (k + 1)],
                    rhs=w2v[:, k, :],
                    start=(k == 0),
                    stop=(k == HCH - 1),
                )

            # next step's Z injects + remaining Z weave fill the argmax-tail window
            halves_next = injects(t + 1)
            for kk in zwork:
                z_part(j + 1, kk, DCH // 2, DCH)

            # argmax tail on DVE: +b2 fused into the psum evacuation
            lslice = lg_accum[:, s * O : (s + 1) * O]
            nc.vector.tensor_add(lslice, lg[:], b2_sb[:])
            mx = small.tile([B, 8], F32, tag="mx")
            nc.vector.max(mx[:], lslice)
            oh = small.tile([B, O], F16, tag="oh")
            nc.vector.tensor_scalar(
                oh[:], lslice, mx[:, 0:1], None, op0=AluOp.is_equal
            )
            ohT = small.tile([O, B], F16, tag="ohT")
            nc.vector.transpose(ohT[:], oh[:])
            ohT_prev = ohT

            if s == BLK - 1:
                nc.sync.dma_start(
                    out_logits[:, j * BLK : (j + 1) * BLK, :],
                    lg_accum[:].rearrange("p (s c) -> p s c", c=O),
                )
                if t != T - 1:
                    lg_accum = lgpool.tile([B, BLK * O], F32, tag="lgacc")

    nc.compile()
    return nc


def kernel(x, W1, b1, W2, b2):
    global LAST_EXEC_NS
    x = np.ascontiguousarray(np.asarray(x, dtype=np.float32))
    W1 = np.ascontiguousarray(np.asarray(W1, dtype=np.float32))
    b1 = np.ascontiguousarray(np.asarray(b1, dtype=np.float32))
    W2 = np.ascontiguousarray(np.asarray(W2, dtype=np.float32))
    b2 = np.ascontiguousarray(np.asarray(b2, dtype=np.float32))

    if "nc" not in _CACHE:
        _CACHE["nc"] = _build()
    nc = _CACHE["nc"]

    w1x = np.ascontiguousarray(W1[:D])
    w1oh = np.ascontiguousarray(W1[D:]).astype(np.float16)
    b1t = np.ascontiguousarray(b1.reshape(HCH, 128).T)
    b2rep = np.ascontiguousarray(np.tile(b2, (B, 1)))
    ident = np.eye(128, dtype=np.float32)
    oh0 = (np.arange(O)[:, None] == np.zeros(B)[None, :]).astype(np.float16)

    in_maps = []
    for c in range(NCORES):
        xc = x[c * B : (c + 1) * B]                       # [32, 512, 1024]
        xTc = np.ascontiguousarray(xc.transpose(2, 1, 0)).reshape(D, T * B)
        in_maps.append(
            dict(xT=xTc, w1x=w1x, w1oh=w1oh, w2=W2, b1t=b1t, b2rep=b2rep, ident=ident, oh0=oh0)
        )

    import jax
    devs = jax.devices()
    if not any(d.platform != "cpu" for d in devs):
        jax.config.update("jax_platforms", "axon,cpu")

    trace = bool(int(os.environ.get("BIGRAM_TRACE", "0")))
    res = run_bass_kernel_spmd(
        nc, in_maps, core_ids=list(range(NCORES)), trace=trace
    )
    LAST_EXEC_NS = res.exec_time_ns

    logits = np.empty((NCORES * B, T, O), dtype=np.float32)
    for c in range(NCORES):
        logits[c * B : (c + 1) * B] = res.results[c]["out_logits"]
    preds = np.argmax(logits, axis=-1).astype(np.int32)
    return logits, preds
